# revision 19
# baseline (speedup 1.0000x reference)
"""Trainium2 Bass kernel for nn_AttnResBlock (B=16, C=512, A=64, L=1024).

Data-parallel over batch: 8 cores x 2 batches each, weights replicated.
BatchNorm (training mode, stats over (B, L)) needs global batch stats ->
two tiny [128, 8] f32 AllReduces; a same-shape warmup collective at
kernel start absorbs the first-collective setup cost under input DMA.

Precision (rel-err budget 2e-2, measured ~3e-3):
  - x, xT arrive bf16 (halves input DMA); x2 and the output stay f32.
  - attnout + proj matmuls run fp8(e4m3) DoubleRow: 2 contraction
    rows/partition/cycle -> half the matmuls. xTs carries S_A=256 (the
    softmax recip is ~1e-3, below fp8 normal range), wp carries S_W=16;
    1/(S_A*S_W) folds into the x2 epilogue scalar. fp8 noise lands only
    on the attention output, which is small next to the residual.
  - convs + kq + scores stay bf16: fp8 would put ~3% relative noise on
    conv outputs (quantization noise does not average down in
    incoherent sums), blowing the budget.

Schedule:
  - Engine-order stalls sequence the DMA traffic: x/xT(b0) first, conv
    weights issue from the gpsimd queue only after the b0 queries move
    (so they never race the phase-A-critical loads; the old kernel lost
    ~10us to weight traffic queued in front of the BN1 stats DMA).
  - Phase A is software-pipelined across the two local batches: kq(b1)
    runs right after kq(b0) so b1's keys/queries are staged long before
    the PE reaches b1's scores.
  - x is pre-biased into xb = x + bp during the load window, so the x2
    epilogue is one DVE op (psum*s + xb, channel sums via accum_out);
    sums-of-squares ride ACT Square accum_out. BN stats cost no
    dedicated passes.
  - Convs run hc-outer / oc-inner with two PSUM banks (b) per step; the
    last group's epilogue is ~1.5us, and conv2 streams 512-col output
    chunks to HBM as they finish.
"""
import numpy as np

P = 128
B, C, A, L = 16, 512, 64, 1024
NCORES = 8
BL = B // NCORES          # local batches per core
CT = C // P               # 4 channel tiles
LT = L // P               # 8 length tiles
MC = L // 512             # 2 moving chunks
EPS = 1e-5
SM_SCALE = 2.0 / L        # softmax scale: scores/(L/2)
S_W = 16.0                # fp8 scale for wp
S_A = 256.0               # fp8 scale for xTs
CONV1_FP8 = True          # conv1 in fp8 DoubleRow (w1*S_W, h in fp8)
CONV2_FP8 = False         # conv2 stays bf16 (its fp8 noise hits the output directly)
USE_AG = True             # AllGather + local reduce instead of AllReduce
EPS2 = (S_W * S_W if CONV1_FP8 else 1.0) * EPS   # bn2 eps for scaled h2

_CACHE = {}


def _build():
    import concourse.bass as bass
    import concourse.mybir as mybir
    from concourse import bacc
    from concourse.tile import TileContext

    f32 = mybir.dt.float32
    bf16 = mybir.dt.bfloat16
    fp8 = mybir.dt.float8e4
    AF = mybir.ActivationFunctionType
    ALU = mybir.AluOpType
    DR = mybir.MatmulPerfMode.DoubleRow

    nc = bacc.Bacc(num_devices=NCORES)

    x_ext = nc.declare_dram_parameter("x", [BL, C, L], bf16, isOutput=False)
    xT_ext = nc.declare_dram_parameter("xT", [BL, L, C], bf16, isOutput=False)
    wkq_ext = nc.declare_dram_parameter("wkq", [P, CT * 2 * A], bf16, isOutput=False)
    wp_ext = nc.declare_dram_parameter("wp", [P, CT * C], fp8, isOutput=False)
    w1dt = fp8 if CONV1_FP8 else bf16
    w2dt = fp8 if CONV2_FP8 else bf16
    w1_ext = nc.declare_dram_parameter("w1", [P, 3 * CT * C], w1dt, isOutput=False)
    w2_ext = nc.declare_dram_parameter("w2", [P, 3 * CT * C], w2dt, isOutput=False)
    # per-channel vectors packed [P, CT] each: bp b1 b2 g1 be1 g2 be2, then bkq
    pvec_ext = nc.declare_dram_parameter("pvec", [P, 7 * CT + 1], f32, isOutput=False)
    out_ext = nc.declare_dram_parameter("out", [BL, C, L], f32, isOutput=True)

    NOUT = NCORES if USE_AG else 1
    cc0_in = nc.dram_tensor("cc0_in", [P, 2 * CT], f32)
    cc0_out = nc.dram_tensor("cc0_out", [NOUT * P, 2 * CT], f32, addr_space="Shared")
    cc1_in = nc.dram_tensor("cc1_in", [P, 2 * CT], f32)
    cc1_out = nc.dram_tensor("cc1_out", [NOUT * P, 2 * CT], f32, addr_space="Shared")
    cc2_in = nc.dram_tensor("cc2_in", [P, 2 * CT], f32)
    cc2_out = nc.dram_tensor("cc2_out", [NOUT * P, 2 * CT], f32, addr_space="Shared")
    CCKIND = "AllGather" if USE_AG else "AllReduce"
    CCOP = (mybir.AluOpType.bypass if USE_AG else mybir.AluOpType.add)

    rg = [list(range(NCORES))]

    with TileContext(nc) as tc:
        with tc.tile_pool(name="pers", bufs=1) as pers, \
             tc.tile_pool(name="small", bufs=1) as small, \
             tc.tile_pool(name="ostage", bufs=4) as ostage, \
             tc.tile_pool(name="phA", bufs=2) as pab, \
             tc.tile_pool(name="psum", bufs=8, space="PSUM") as psum:

            # ---- kernel-lifetime tiles ----
            x2_sb = pers.tile([P, BL, CT, L], f32)
            x_sb = pers.tile([P, BL, CT, L], bf16)
            wkq_sb = pers.tile([P, CT, 2 * A], bf16)
            wp_sb = pers.tile([P, CT, C], fp8)
            w1_sb = pers.tile([P, 3 * CT, C], w1dt)
            w2_sb = pers.tile([P, 3 * CT, C], w2dt)
            h_sb = pers.tile([P, BL, CT, L + 2], fp8 if CONV1_FP8 else bf16)
            h3_sb = pers.tile([P, BL, CT, L + 2], fp8 if CONV2_FP8 else bf16)
            h2_sb = pers.tile([P, BL, CT, L], bf16)
            keys_sb = pers.tile([P, BL, L], bf16)       # keys 0:A, queries A:2A
            queries_sb = pers.tile([P, BL, L], bf16)    # queries at base 0

            # warmup collective: absorbs the first-collective setup cost
            nc.gpsimd.collective_compute(
                CCKIND, CCOP, replica_groups=rg,
                ins=[cc0_in[:].opt()], outs=[cc0_out[:].opt()])

            pvec_sb = small.tile([P, 7 * CT + 1], f32, tag="pvec")
            nc.gpsimd.dma_start(out=pvec_sb[:], in_=pvec_ext[:])
            bp_sb = pvec_sb[:, 0 * CT:1 * CT]
            b1_sb = pvec_sb[:, 1 * CT:2 * CT]
            b2_sb = pvec_sb[:, 2 * CT:3 * CT]
            g1_sb = pvec_sb[:, 3 * CT:4 * CT]
            be1_sb = pvec_sb[:, 4 * CT:5 * CT]
            g2_sb = pvec_sb[:, 5 * CT:6 * CT]
            be2_sb = pvec_sb[:, 6 * CT:7 * CT]
            bkq_sb = pvec_sb[:, 7 * CT:7 * CT + 1]   # [bk; bq]

            # phase-A-critical loads on the sync queue: wkq then x
            nc.sync.dma_start(out=wkq_sb[:],
                              in_=wkq_ext[:].rearrange("p (ct a) -> p ct a", ct=CT))
            for b in range(BL):
                for q in range(L // 256):
                    for ct in range(CT):
                        nc.sync.dma_start(
                            out=x_sb[:, b, ct, q * 256:(q + 1) * 256],
                            in_=x_ext[b, ct * P:(ct + 1) * P, q * 256:(q + 1) * 256])
            # xT(b0) early on gpsimd; xT(b1) + conv weights issue later from
            # the same queue, after it stalls on the b0 queries move
            xT_tiles = [pab.tile([P, LT, C], bf16, tag="xT", name=f"xT{_b}")
                        for _b in range(BL)]
            nc.gpsimd.dma_start(out=xT_tiles[0][:],
                                in_=xT_ext[0].rearrange("(lc p) c -> p lc c", p=P))

            g1g_sb = small.tile([P, NOUT, 2 * CT], f32, tag="g1g")
            g2g_sb = small.tile([P, NOUT, 2 * CT], f32, tag="g2g")
            ccin1_sb = small.tile([P, 2 * CT], f32, tag="ccin1")
            ccout1_sb = small.tile([P, 2 * CT], f32, tag="ccout1")
            ccin2_sb = small.tile([P, 2 * CT], f32, tag="ccin2")
            ccout2_sb = small.tile([P, 2 * CT], f32, tag="ccout2")
            # per-chunk stat accumulators [P, ct, 2*b+chunk]
            m1a = small.tile([P, CT, 2 * BL], f32, tag="m1a")   # sum(x2)
            m2a = small.tile([P, CT, 2 * BL], f32, tag="m2a")   # sum(x2^2)
            n1a = small.tile([P, CT, 2 * BL], f32, tag="n1a")   # sum(h2)
            n2a = small.tile([P, CT, 2 * BL], f32, tag="n2a")   # sum(h2^2)
            scale1 = small.tile([P, CT], f32, tag="scale1")
            bias1 = small.tile([P, CT], f32, tag="bias1")
            scale2 = small.tile([P, CT], f32, tag="scale2")
            bias2 = small.tile([P, CT], f32, tag="bias2")
            eps_sb = small.tile([P, 2], f32, tag="eps")
            nc.vector.memset(eps_sb[:, 0:1], EPS)
            nc.vector.memset(eps_sb[:, 1:2], EPS2)

            # conv padding zeros (columns 0 and L+1 of every (b, ct) row)
            nc.vector.memset(h_sb[:, :, :, 0], 0.0)
            nc.vector.memset(h_sb[:, :, :, L + 1], 0.0)
            nc.vector.memset(h3_sb[:, :, :, 0], 0.0)
            nc.vector.memset(h3_sb[:, :, :, L + 1], 0.0)

            # pre-warm ACT function tables (only 4 funcs used all kernel)
            warm = small.tile([P, 1], f32, tag="warm")
            for fn in (AF.Square, AF.Sqrt, AF.Relu, AF.Exp):
                nc.scalar.activation(out=warm[:], in_=eps_sb[:, 0:1], func=fn)

            def stats_allreduce(ccin_dram, ccout_dram, ccin_sb, ccred_sb,
                                gath_sb=None):
                nc.gpsimd.dma_start(out=ccin_dram[:], in_=ccin_sb[:])
                nc.gpsimd.collective_compute(
                    CCKIND, CCOP, replica_groups=rg,
                    ins=[ccin_dram[:].opt()], outs=[ccout_dram[:].opt()])
                if USE_AG:
                    # gathered [P, r, 2CT]; tree-reduce the rank dim on DVE
                    nc.gpsimd.dma_start(
                        out=gath_sb[:],
                        in_=ccout_dram[:].rearrange("(r p) c -> p r c", p=P))
                    nc.vector.tensor_tensor(
                        out=gath_sb[:, 0:4, :], in0=gath_sb[:, 0:4, :],
                        in1=gath_sb[:, 4:8, :], op=ALU.add)
                    nc.vector.tensor_tensor(
                        out=gath_sb[:, 0:2, :], in0=gath_sb[:, 0:2, :],
                        in1=gath_sb[:, 2:4, :], op=ALU.add)
                    nc.vector.tensor_tensor(
                        out=ccred_sb[:], in0=gath_sb[:, 0, :],
                        in1=gath_sb[:, 1, :], op=ALU.add)
                else:
                    nc.gpsimd.dma_start(out=ccred_sb[:], in_=ccout_dram[:])

            def pack_stats(msum, sqsum, ccin_sb):
                nc.vector.tensor_reduce(out=ccin_sb[:, 0:CT], in_=msum[:],
                                        axis=mybir.AxisListType.X, op=ALU.add)
                nc.vector.tensor_reduce(out=ccin_sb[:, CT:2 * CT], in_=sqsum[:],
                                        axis=mybir.AxisListType.X, op=ALU.add)

            def bn_post(ccout_sb, g_sb, be_sb, scale_t, bias_t, tag, eps_ap):
                mgx = small.tile([P, 2 * CT], f32, tag=tag + "mgx")
                nc.vector.tensor_scalar_mul(out=mgx[:], in0=ccout_sb[:],
                                            scalar1=1.0 / (B * L))
                mg = mgx[:, 0:CT]
                ex2 = mgx[:, CT:2 * CT]
                nvar = small.tile([P, CT], f32, tag=tag + "nv")
                # nvar = mean^2 - E[x^2] = -var
                nc.vector.tensor_tensor(out=nvar[:], in0=mg, in1=mg, op=ALU.mult)
                nc.vector.tensor_tensor(out=nvar[:], in0=nvar[:], in1=ex2, op=ALU.subtract)
                sd = small.tile([P, CT], f32, tag=tag + "sd")
                nc.scalar.activation(out=sd[:], in_=nvar[:], func=AF.Sqrt,
                                     scale=-1.0, bias=eps_ap)
                rstd = small.tile([P, CT], f32, tag=tag + "rstd")
                nc.vector.reciprocal(out=rstd[:], in_=sd[:])
                nc.vector.tensor_tensor(out=scale_t[:], in0=rstd[:], in1=g_sb[:], op=ALU.mult)
                tmp = small.tile([P, CT], f32, tag=tag + "tmp")
                nc.vector.tensor_tensor(out=tmp[:], in0=mg, in1=scale_t[:], op=ALU.mult)
                nc.vector.tensor_tensor(out=bias_t[:], in0=be_sb[:], in1=tmp[:], op=ALU.subtract)

            # ---------------- Phase A: attention ----------------
            # kq for BOTH batches first (pipelines b1's staging under b0)
            for b in range(BL):
                for mc in range(MC):
                    ms = slice(mc * 512, (mc + 1) * 512)
                    kps = psum.tile([P, 512], f32, tag="ps")
                    for ct in range(CT):
                        nc.tensor.matmul(
                            out=kps[:],
                            lhsT=wkq_sb[:, ct, :],
                            rhs=x_sb[:, b, ct, ms],
                            start=(ct == 0), stop=(ct == CT - 1))
                    # rows 0:64 keys+bk, 64:128 queries+bq (one DVE op)
                    nc.vector.tensor_scalar_add(out=keys_sb[:, b, ms],
                                                in0=kps[:], scalar1=bkq_sb)
                # queries to partition base 0 (SBUF->SBUF remap)
                nc.gpsimd.dma_start(out=queries_sb[0:A, b, :],
                                    in_=keys_sb[A:2 * A, b, :])
                if b == 0:
                    # gpsimd reaches here only after the b0 queries move:
                    # xT(b1) + conv weights now load behind phase-A traffic
                    nc.gpsimd.dma_start(out=xT_tiles[1][:],
                                        in_=xT_ext[1].rearrange("(lc p) c -> p lc c", p=P))
                    nc.gpsimd.dma_start(out=w1_sb[:],
                                        in_=w1_ext[:].rearrange("p (kc c) -> p kc c", c=C))
                    nc.gpsimd.dma_start(out=w2_sb[:],
                                        in_=w2_ext[:].rearrange("p (kc c) -> p kc c", c=C))
                    nc.gpsimd.dma_start(out=wp_sb[:],
                                        in_=wp_ext[:].rearrange("p (ct o) -> p ct o", ct=CT))

            # per-b stage emitters; interleaved b0/b1 for engine balance
            e_t, rsp_t, rcp_t, xTs_t, ao_t = {}, {}, {}, {}, {}

            def stage_scores(b):
                xT_sb = xT_tiles[b]
                e_sb = pab.tile([P, LT, L], fp8, tag="e", name=f"e{b}")
                rsp = pab.tile([P, LT, MC], f32, tag="rsp", name=f"rsp{b}")
                rcp = pab.tile([P, LT], f32, tag="rcp", name=f"rcp{b}")
                xTs = pab.tile([P, LT, C], fp8, tag="xTs", name=f"xTs{b}")
                e_t[b], rsp_t[b], rcp_t[b], xTs_t[b] = e_sb, rsp, rcp, xTs
                for lc in range(LT):
                    for mc in range(MC):
                        sps = psum.tile([P, 512], f32, tag="ps")
                        nc.tensor.matmul(
                            out=sps[:],
                            lhsT=keys_sb[0:A, b, lc * P:(lc + 1) * P],
                            rhs=queries_sb[0:A, b, mc * 512:(mc + 1) * 512],
                            start=True, stop=True)
                        # exp -> fp8, free row sums via the ACT accumulator
                        nc.scalar.activation(
                            out=e_sb[:, lc, mc * 512:(mc + 1) * 512],
                            in_=sps[:], func=AF.Exp, scale=SM_SCALE,
                            accum_out=rsp[:, lc, mc:mc + 1])
                    nc.vector.scalar_tensor_tensor(
                        out=rcp[:, lc:lc + 1], in0=rsp[:, lc, 0:1],
                        scalar=1.0, in1=rsp[:, lc, 1:2],
                        op0=ALU.mult, op1=ALU.add)
                    nc.vector.reciprocal(out=rcp[:, lc:lc + 1],
                                         in_=rcp[:, lc:lc + 1])
                    # xTs[l, c] = xT[l, c] * (S_A / rowsum[l]) -> fp8
                    nc.vector.tensor_scalar(out=xTs[:, lc, :],
                                            in0=xT_sb[:, lc, :],
                                            scalar1=rcp[:, lc:lc + 1],
                                            scalar2=S_A,
                                            op0=ALU.mult, op1=ALU.mult)

            def stage_attnout(b):
                e_sb, xTs = e_t[b], xTs_t[b]
                ao_sb = pab.tile([P, CT, L], fp8, tag="ao", name=f"ao{b}")
                ao_t[b] = ao_sb
                for cc in range(CT):
                    for mc in range(MC):
                        ms = slice(mc * 512, (mc + 1) * 512)
                        aps = psum.tile([P, 512], f32, tag="ps")
                        for lcp in range(LT // 2):
                            nc.tensor.matmul(
                                out=aps[:],
                                lhsT=xTs[:, 2 * lcp:2 * lcp + 2, cc * P:(cc + 1) * P],
                                rhs=e_sb[:, 2 * lcp:2 * lcp + 2, ms],
                                start=(lcp == 0), stop=(lcp == LT // 2 - 1),
                                perf_mode=DR)
                        # b0 casts on DVE, b1 casts on ACT (wave balance)
                        if b == 0:
                            nc.vector.tensor_copy(out=ao_sb[:, cc, ms], in_=aps[:])
                        else:
                            nc.scalar.activation(out=ao_sb[:, cc, ms],
                                                 in_=aps[:], func=AF.Copy)

            def stage_proj(b):
                ao_sb = ao_t[b]
                for oc in range(CT):
                    for mc in range(MC):
                        ms = slice(mc * 512, (mc + 1) * 512)
                        pps = psum.tile([P, 512], f32, tag="ps")
                        for cp in range(CT // 2):
                            nc.tensor.matmul(
                                out=pps[:],
                                lhsT=wp_sb[:, 2 * cp:2 * cp + 2, oc * P:(oc + 1) * P],
                                rhs=ao_sb[:, 2 * cp:2 * cp + 2, ms],
                                start=(cp == 0), stop=(cp == CT // 2 - 1),
                                perf_mode=DR)
                        # x2' = psum/(S_W*S_A) + x  (bp deferred: BN1 is
                        # invariant to per-channel constants; bp rejoins in
                        # the conv2 epilogue via b2+bp)
                        nc.vector.scalar_tensor_tensor(
                            out=x2_sb[:, b, oc, ms], in0=pps[:],
                            scalar=1.0 / (S_W * S_A), in1=x_sb[:, b, oc, ms],
                            op0=ALU.mult, op1=ALU.add,
                            accum_out=m1a[:, oc, 2 * b + mc:2 * b + mc + 1])
                        sqs = ostage.tile([P, 512], f32, tag="sqs")
                        nc.scalar.activation(
                            out=sqs[:], in_=x2_sb[:, b, oc, ms], func=AF.Square,
                            accum_out=m2a[:, oc, 2 * b + mc:2 * b + mc + 1])

            stage_scores(0)
            stage_attnout(0)
            stage_scores(1)
            stage_proj(0)
            stage_attnout(1)
            stage_proj(1)

            # ---------------- BN1 + conv1 ----------------
            pack_stats(m1a, m2a, ccin1_sb)
            stats_allreduce(cc1_in, cc1_out, ccin1_sb, ccout1_sb, g1g_sb)
            for fn in (AF.Sqrt, AF.Relu):   # re-warm while the mesh runs
                nc.scalar.activation(out=warm[:], in_=eps_sb[:, 0:1], func=fn)
            bn_post(ccout1_sb, g1_sb, be1_sb, scale1, bias1, "p1", eps_sb[:, 0:1])

            # h = relu(bn1(x2)), padded; first window split out so conv1's
            # first accumulation group unblocks after ~0.6us of relu
            for b in range(BL):
                for ct in range(CT):
                    if b == 0:
                        nc.scalar.activation(out=h_sb[:, b, ct, 1:515],
                                             in_=x2_sb[:, b, ct, 0:514],
                                             func=AF.Relu,
                                             scale=scale1[:, ct:ct + 1],
                                             bias=bias1[:, ct:ct + 1])
                        nc.scalar.activation(out=h_sb[:, b, ct, 515:L + 1],
                                             in_=x2_sb[:, b, ct, 514:L],
                                             func=AF.Relu,
                                             scale=scale1[:, ct:ct + 1],
                                             bias=bias1[:, ct:ct + 1])
                    else:
                        nc.scalar.activation(out=h_sb[:, b, ct, 1:L + 1],
                                             in_=x2_sb[:, b, ct, :],
                                             func=AF.Relu,
                                             scale=scale1[:, ct:ct + 1],
                                             bias=bias1[:, ct:ct + 1])

            # conv1: hc-outer, oc-inner, 2 psum banks (b)
            for hc in range(MC):
                for oc in range(CT):
                    cps = [psum.tile([P, 512], f32, tag="ps", name=f"c1ps{hc}_{oc}_{_j}")
                           for _j in range(BL)]
                    if CONV1_FP8:
                        for cp in range(CT // 2):
                            for k in range(3):
                                w_ap = w1_sb[:, k * CT + 2 * cp:k * CT + 2 * cp + 2,
                                             oc * P:(oc + 1) * P]
                                for b in range(BL):
                                    nc.tensor.matmul(
                                        out=cps[b][:], lhsT=w_ap,
                                        rhs=h_sb[:, b, 2 * cp:2 * cp + 2,
                                                 hc * 512 + k:hc * 512 + k + 512],
                                        start=(cp == 0 and k == 0),
                                        stop=(cp == CT // 2 - 1 and k == 2),
                                        perf_mode=DR)
                    else:
                        for ct in range(CT):
                            for k in range(3):
                                w_ap = w1_sb[:, k * CT + ct, oc * P:(oc + 1) * P]
                                for b in range(BL):
                                    nc.tensor.matmul(
                                        out=cps[b][:], lhsT=w_ap,
                                        rhs=h_sb[:, b, ct, hc * 512 + k:hc * 512 + k + 512],
                                        start=(ct == 0 and k == 0),
                                        stop=(ct == CT - 1 and k == 2))
                    for b in range(BL):
                        hs = slice(hc * 512, (hc + 1) * 512)
                        nc.vector.tensor_scalar(
                            out=h2_sb[:, b, oc, hs], in0=cps[b][:],
                            scalar1=b1_sb[:, oc:oc + 1], scalar2=0.0,
                            op0=ALU.add, op1=ALU.add,
                            accum_out=n1a[:, oc, 2 * b + hc:2 * b + hc + 1])
                        sqs = ostage.tile([P, 512], f32, tag="sqs")
                        nc.scalar.activation(
                            out=sqs[:], in_=h2_sb[:, b, oc, hs], func=AF.Square,
                            accum_out=n2a[:, oc, 2 * b + hc:2 * b + hc + 1])

            # ---------------- BN2 + conv2 ----------------
            pack_stats(n1a, n2a, ccin2_sb)
            stats_allreduce(cc2_in, cc2_out, ccin2_sb, ccout2_sb, g2g_sb)
            warm2 = (AF.Sqrt, AF.Relu, AF.Identity) if CONV2_FP8 else (AF.Sqrt, AF.Relu)
            for fn in warm2:                # re-warm while the mesh runs
                nc.scalar.activation(out=warm[:], in_=eps_sb[:, 0:1], func=fn)
            bn_post(ccout2_sb, g2_sb, be2_sb, scale2, bias2, "p2", eps_sb[:, 1:2])

            # h3 = relu(bn2(h2))
            for b in range(BL):
                for ct in range(CT):
                    if b == 0:
                        nc.scalar.activation(out=h3_sb[:, b, ct, 1:515],
                                             in_=h2_sb[:, b, ct, 0:514],
                                             func=AF.Relu,
                                             scale=scale2[:, ct:ct + 1],
                                             bias=bias2[:, ct:ct + 1])
                        nc.scalar.activation(out=h3_sb[:, b, ct, 515:L + 1],
                                             in_=h2_sb[:, b, ct, 514:L],
                                             func=AF.Relu,
                                             scale=scale2[:, ct:ct + 1],
                                             bias=bias2[:, ct:ct + 1])
                    else:
                        nc.scalar.activation(out=h3_sb[:, b, ct, 1:L + 1],
                                             in_=h2_sb[:, b, ct, :],
                                             func=AF.Relu,
                                             scale=scale2[:, ct:ct + 1],
                                             bias=bias2[:, ct:ct + 1])

            # conv2 + b2 + residual -> out, streaming 512-col chunks to HBM
            for hc in range(MC):
                for oc in range(CT):
                    cps = [psum.tile([P, 512], f32, tag="ps", name=f"c2ps{hc}_{oc}_{_j}")
                           for _j in range(BL)]
                    if CONV2_FP8:
                        for cp in range(CT // 2):
                            for k in range(3):
                                w_ap = w2_sb[:, k * CT + 2 * cp:k * CT + 2 * cp + 2,
                                             oc * P:(oc + 1) * P]
                                for b in range(BL):
                                    nc.tensor.matmul(
                                        out=cps[b][:], lhsT=w_ap,
                                        rhs=h3_sb[:, b, 2 * cp:2 * cp + 2,
                                                  hc * 512 + k:hc * 512 + k + 512],
                                        start=(cp == 0 and k == 0),
                                        stop=(cp == CT // 2 - 1 and k == 2),
                                        perf_mode=DR)
                    else:
                        for ct in range(CT):
                            for k in range(3):
                                w_ap = w2_sb[:, k * CT + ct, oc * P:(oc + 1) * P]
                                for b in range(BL):
                                    nc.tensor.matmul(
                                        out=cps[b][:], lhsT=w_ap,
                                        rhs=h3_sb[:, b, ct, hc * 512 + k:hc * 512 + k + 512],
                                        start=(ct == 0 and k == 0),
                                        stop=(ct == CT - 1 and k == 2))
                    for b in range(BL):
                        hs = slice(hc * 512, (hc + 1) * 512)
                        og = ostage.tile([P, 512], f32, tag="og")
                        if CONV2_FP8:
                            ogt = ostage.tile([P, 512], f32, tag="ogt")
                            nc.scalar.activation(
                                out=ogt[:], in_=cps[b][:], func=AF.Identity,
                                scale=1.0 / S_W, bias=b2_sb[:, oc:oc + 1])
                            nc.vector.tensor_tensor(
                                out=og[:], in0=ogt[:],
                                in1=x2_sb[:, b, oc, hs], op=ALU.add)
                        else:
                            nc.vector.scalar_tensor_tensor(
                                out=og[:], in0=cps[b][:],
                                scalar=b2_sb[:, oc:oc + 1],
                                in1=x2_sb[:, b, oc, hs],
                                op0=ALU.add, op1=ALU.add)
                        nc.sync.dma_start(
                            out=out_ext[b, oc * P:(oc + 1) * P, hs], in_=og[:])

    nc.compile()
    return nc


def _get_nc():
    if "nc" not in _CACHE:
        _CACHE["nc"] = _build()
    return _CACHE["nc"]


def _prep_in_maps(inputs):
    import ml_dtypes
    f = np.float32
    bf = ml_dtypes.bfloat16
    f8 = ml_dtypes.float8_e4m3
    x = np.ascontiguousarray(inputs["x"], dtype=f)

    def vec_pct(v):
        # (C,) -> [P, CT] with channel c = ct*P + p at [p, ct]
        return np.asarray(v, dtype=f).reshape(CT, P).T

    pvec = np.concatenate(
        [vec_pct(inputs["bp"]),
         vec_pct(inputs["b1"]) * (S_W if CONV1_FP8 else 1.0),
         vec_pct(inputs["b2"]) + vec_pct(inputs["bp"]),
         vec_pct(inputs["g1"]), vec_pct(inputs["be1"]),
         vec_pct(inputs["g2"]), vec_pct(inputs["be2"]),
         np.concatenate([inputs["bk"], inputs["bq"]]).reshape(P, 1).astype(f)],
        axis=1)

    def swiz2(w):  # [C, X] -> [P, CT*X] partition-major
        X = w.shape[1]
        return np.ascontiguousarray(
            w.reshape(CT, P, X).transpose(1, 0, 2).reshape(P, CT * X))

    def swiz3(w):  # [3, C, C] (k, i, o) -> [P, 3*CT*C] with cols (k*CT+ct)*C+o
        return np.ascontiguousarray(
            w.reshape(3, CT, P, C).transpose(2, 0, 1, 3).reshape(P, 3 * CT * C))

    shared = {
        "wkq": swiz2(np.concatenate([inputs["Wk"].T, inputs["Wq"].T], axis=1).astype(bf)),
        "wp": swiz2((inputs["Wp"].T * S_W).astype(f8)),
        "w1": swiz3((np.transpose(inputs["W1"], (2, 1, 0)) * S_W).astype(f8)
                    if CONV1_FP8 else
                    np.transpose(inputs["W1"], (2, 1, 0)).astype(bf)),
        "w2": swiz3((np.transpose(inputs["W2"], (2, 1, 0)) * S_W).astype(f8)
                    if CONV2_FP8 else
                    np.transpose(inputs["W2"], (2, 1, 0)).astype(bf)),
        "pvec": np.ascontiguousarray(pvec, dtype=f),
    }
    in_maps = []
    for i in range(NCORES):
        xl = np.ascontiguousarray(x[i * BL:(i + 1) * BL])
        xTl = np.ascontiguousarray(np.transpose(xl, (0, 2, 1)).astype(bf))
        m = {"x": xl.astype(bf), "xT": xTl}
        m.update(shared)
        in_maps.append(m)
    return in_maps


def kernel(**inputs) -> np.ndarray:
    from concourse import bass_utils
    nc = _get_nc()
    in_maps = _prep_in_maps(inputs)
    res = bass_utils.run_bass_kernel_spmd(nc, in_maps, list(range(NCORES)))
    return np.concatenate([r["out"] for r in res.results], axis=0)


# revision 27
# speedup vs baseline: 1.0859x; 1.0859x over previous
"""Trainium2 Bass kernel for nn_AttnResBlock (B=16, C=512, A=64, L=1024).

Data-parallel over batch: 8 cores x 2 batches each, weights replicated.
BatchNorm (training mode, stats over (B, L)) needs global batch stats ->
two tiny [128, 8] f32 AllReduces; a same-shape warmup collective at
kernel start absorbs the first-collective setup cost under input DMA.

Precision (rel-err budget 2e-2, measured ~3e-3):
  - x, xT arrive bf16 (halves input DMA); x2 and the output stay f32.
  - attnout + proj matmuls run fp8(e4m3) DoubleRow: 2 contraction
    rows/partition/cycle -> half the matmuls. xTs carries S_A=256 (the
    softmax recip is ~1e-3, below fp8 normal range), wp carries S_W=16;
    1/(S_A*S_W) folds into the x2 epilogue scalar. fp8 noise lands only
    on the attention output, which is small next to the residual.
  - convs + kq + scores stay bf16: fp8 would put ~3% relative noise on
    conv outputs (quantization noise does not average down in
    incoherent sums), blowing the budget.

Schedule:
  - Engine-order stalls sequence the DMA traffic: x/xT(b0) first, conv
    weights issue from the gpsimd queue only after the b0 queries move
    (so they never race the phase-A-critical loads; the old kernel lost
    ~10us to weight traffic queued in front of the BN1 stats DMA).
  - Phase A is software-pipelined across the two local batches: kq(b1)
    runs right after kq(b0) so b1's keys/queries are staged long before
    the PE reaches b1's scores.
  - x is pre-biased into xb = x + bp during the load window, so the x2
    epilogue is one DVE op (psum*s + xb, channel sums via accum_out);
    sums-of-squares ride ACT Square accum_out. BN stats cost no
    dedicated passes.
  - Convs run hc-outer / oc-inner with two PSUM banks (b) per step; the
    last group's epilogue is ~1.5us, and conv2 streams 512-col output
    chunks to HBM as they finish.
"""
import numpy as np

P = 128
B, C, A, L = 16, 512, 64, 1024
NCORES = 8
BL = B // NCORES          # local batches per core
CT = C // P               # 4 channel tiles
LT = L // P               # 8 length tiles
MC = L // 512             # 2 moving chunks
EPS = 1e-5
SM_SCALE = 2.0 / L        # softmax scale: scores/(L/2)
S_W = 16.0                # fp8 scale for wp
S_A = 256.0               # fp8 scale for xTs
CONV1_FP8 = True          # conv1 in fp8 DoubleRow (w1*S_W, h in fp8)
CONV2_FP8 = False         # conv2 stays bf16 (its fp8 noise hits the output directly)
USE_AG = True             # AllGather + local reduce instead of AllReduce
EPS2 = (S_W * S_W if CONV1_FP8 else 1.0) * EPS   # bn2 eps for scaled h2

_CACHE = {}


def _build():
    import concourse.bass as bass
    import concourse.mybir as mybir
    from concourse import bacc
    from concourse.tile import TileContext

    f32 = mybir.dt.float32
    bf16 = mybir.dt.bfloat16
    fp8 = mybir.dt.float8e4
    AF = mybir.ActivationFunctionType
    ALU = mybir.AluOpType
    DR = mybir.MatmulPerfMode.DoubleRow

    nc = bacc.Bacc(num_devices=NCORES)

    x_ext = nc.declare_dram_parameter("x", [BL, C, L], bf16, isOutput=False)
    xT_ext = nc.declare_dram_parameter("xT", [BL, L, C], bf16, isOutput=False)
    wkq_ext = nc.declare_dram_parameter("wkq", [P, CT * 2 * A], bf16, isOutput=False)
    wp_ext = nc.declare_dram_parameter("wp", [P, CT * C], fp8, isOutput=False)
    w1dt = fp8 if CONV1_FP8 else bf16
    w2dt = fp8 if CONV2_FP8 else bf16
    w1_ext = nc.declare_dram_parameter("w1", [P, 3 * CT * C], w1dt, isOutput=False)
    w2_ext = nc.declare_dram_parameter("w2", [P, 3 * CT * C], w2dt, isOutput=False)
    # per-channel vectors packed [P, CT] each: bp b1 b2 g1 be1 g2 be2, then bkq
    pvec_ext = nc.declare_dram_parameter("pvec", [P, 7 * CT + 1], f32, isOutput=False)
    out_ext = nc.declare_dram_parameter("out", [BL, C, L], f32, isOutput=True)

    NOUT = NCORES if USE_AG else 1
    cc0_in = nc.dram_tensor("cc0_in", [P, 1], f32)
    cc0_out = nc.dram_tensor("cc0_out", [NOUT * P, 1], f32, addr_space="Shared")
    cc1_in = nc.dram_tensor("cc1_in", [P, 2 * CT], f32)
    cc1_out = nc.dram_tensor("cc1_out", [NOUT * P, 2 * CT], f32, addr_space="Shared")
    cc2_in = nc.dram_tensor("cc2_in", [P, 2 * CT], f32)
    cc2_out = nc.dram_tensor("cc2_out", [NOUT * P, 2 * CT], f32, addr_space="Shared")
    CCKIND = "AllGather" if USE_AG else "AllReduce"
    CCOP = (mybir.AluOpType.bypass if USE_AG else mybir.AluOpType.add)

    rg = [list(range(NCORES))]

    with TileContext(nc) as tc:
        with tc.tile_pool(name="pers", bufs=1) as pers, \
             tc.tile_pool(name="small", bufs=1) as small, \
             tc.tile_pool(name="ostage", bufs=4) as ostage, \
             tc.tile_pool(name="phA", bufs=2) as pab, \
             tc.tile_pool(name="psum", bufs=8, space="PSUM") as psum:

            # ---- kernel-lifetime tiles ----
            x2_sb = pers.tile([P, BL, CT, L], f32)
            x_sb = pers.tile([P, BL, CT, L], bf16)
            wkq_sb = pers.tile([P, CT, 2 * A], bf16)
            wp_sb = pers.tile([P, CT, C], fp8)
            w1_sb = pers.tile([P, 3 * CT, C], w1dt)
            w2_sb = pers.tile([P, 3 * CT, C], w2dt)
            HPAD = 2048 if CONV1_FP8 else L + 2   # aligned ct-stride for DR
            h_sb = pers.tile([P, BL, CT, HPAD], fp8 if CONV1_FP8 else bf16)
            h3_sb = pers.tile([P, BL, CT, L + 2], fp8 if CONV2_FP8 else bf16)
            h2_sb = pers.tile([P, BL, CT, L], bf16)
            keys_sb = pers.tile([P, BL, L], bf16)       # keys 0:A, queries A:2A
            queries_sb = pers.tile([P, BL, L], bf16)    # queries at base 0

            # warmup collective: absorbs the first-collective setup cost
            nc.gpsimd.collective_compute(
                CCKIND, CCOP, replica_groups=rg,
                ins=[cc0_in[:].opt()], outs=[cc0_out[:].opt()])

            pvec_sb = small.tile([P, 7 * CT + 1], f32, tag="pvec")
            nc.gpsimd.dma_start(out=pvec_sb[:], in_=pvec_ext[:])
            bp_sb = pvec_sb[:, 0 * CT:1 * CT]
            b1_sb = pvec_sb[:, 1 * CT:2 * CT]
            b2_sb = pvec_sb[:, 2 * CT:3 * CT]
            g1_sb = pvec_sb[:, 3 * CT:4 * CT]
            be1_sb = pvec_sb[:, 4 * CT:5 * CT]
            g2_sb = pvec_sb[:, 5 * CT:6 * CT]
            be2_sb = pvec_sb[:, 6 * CT:7 * CT]
            bkq_sb = pvec_sb[:, 7 * CT:7 * CT + 1]   # [bk; bq]

            # phase-A-critical loads on the sync queue: wkq then x
            nc.sync.dma_start(out=wkq_sb[:],
                              in_=wkq_ext[:].rearrange("p (ct a) -> p ct a", ct=CT))
            for b in range(BL):
                for mc in range(MC):
                    for ct in range(CT):
                        nc.sync.dma_start(
                            out=x_sb[:, b, ct, mc * 512:(mc + 1) * 512],
                            in_=x_ext[b, ct * P:(ct + 1) * P, mc * 512:(mc + 1) * 512])
            # xT(b0) early on gpsimd; xT(b1) + conv weights issue later from
            # the same queue, after it stalls on the b0 queries move
            xT_tiles = [pab.tile([P, LT, C], bf16, tag="xT", name=f"xT{_b}")
                        for _b in range(BL)]

            g1g_sb = small.tile([P, NOUT, 2 * CT], f32, tag="g1g")
            g2g_sb = small.tile([P, NOUT, 2 * CT], f32, tag="g2g")
            ccin1_sb = small.tile([P, 2 * CT], f32, tag="ccin1")
            ccout1_sb = small.tile([P, 2 * CT], f32, tag="ccout1")
            ccin2_sb = small.tile([P, 2 * CT], f32, tag="ccin2")
            ccout2_sb = small.tile([P, 2 * CT], f32, tag="ccout2")
            # per-chunk stat accumulators [P, ct, 2*b+chunk]
            m1a = small.tile([P, CT, 2 * BL], f32, tag="m1a")   # sum(x2)
            m2a = small.tile([P, CT, 2 * BL], f32, tag="m2a")   # sum(x2^2)
            n1a = small.tile([P, CT, 2 * BL], f32, tag="n1a")   # sum(h2)
            n2a = small.tile([P, CT, 2 * BL], f32, tag="n2a")   # sum(h2^2)
            scale1 = small.tile([P, CT], f32, tag="scale1")
            bias1 = small.tile([P, CT], f32, tag="bias1")
            scale2 = small.tile([P, CT], f32, tag="scale2")
            bias2 = small.tile([P, CT], f32, tag="bias2")
            eps_sb = small.tile([P, 2], f32, tag="eps")
            nc.vector.memset(eps_sb[:, 0:1], EPS)
            nc.vector.memset(eps_sb[:, 1:2], EPS2)

            # conv padding zeros (columns 0 and L+1 of every (b, ct) row)
            nc.vector.memset(h_sb[:, :, :, 0], 0.0)
            nc.vector.memset(h_sb[:, :, :, L + 1], 0.0)
            nc.vector.memset(h3_sb[:, :, :, 0], 0.0)
            nc.vector.memset(h3_sb[:, :, :, L + 1], 0.0)

            # pre-warm ACT function tables (only 4 funcs used all kernel)
            warm = small.tile([P, 1], f32, tag="warm")
            for fn in (AF.Square, AF.Sqrt, AF.Relu, AF.Exp):
                nc.scalar.activation(out=warm[:], in_=eps_sb[:, 0:1], func=fn)

            def stats_allreduce(ccin_dram, ccout_dram, ccin_sb, ccred_sb,
                                gath_sb=None):
                nc.gpsimd.dma_start(out=ccin_dram[:], in_=ccin_sb[:])
                nc.gpsimd.collective_compute(
                    CCKIND, CCOP, replica_groups=rg,
                    ins=[ccin_dram[:].opt()], outs=[ccout_dram[:].opt()])
                if USE_AG:
                    # gathered [P, r, 2CT]; readback + tree-reduce both on DVE
                    nc.scalar.dma_start(
                        out=gath_sb[:],
                        in_=ccout_dram[:].rearrange("(r p) c -> p r c", p=P))
                    nc.vector.tensor_tensor(
                        out=gath_sb[:, 0:4, :], in0=gath_sb[:, 0:4, :],
                        in1=gath_sb[:, 4:8, :], op=ALU.add)
                    nc.vector.tensor_tensor(
                        out=gath_sb[:, 0:2, :], in0=gath_sb[:, 0:2, :],
                        in1=gath_sb[:, 2:4, :], op=ALU.add)
                    nc.vector.tensor_tensor(
                        out=ccred_sb[:], in0=gath_sb[:, 0, :],
                        in1=gath_sb[:, 1, :], op=ALU.add)
                else:
                    nc.gpsimd.dma_start(out=ccred_sb[:], in_=ccout_dram[:])

            def pack_stats(msum, sqsum, ccin_sb):
                nc.vector.tensor_reduce(out=ccin_sb[:, 0:CT], in_=msum[:],
                                        axis=mybir.AxisListType.X, op=ALU.add)
                nc.vector.tensor_reduce(out=ccin_sb[:, CT:2 * CT], in_=sqsum[:],
                                        axis=mybir.AxisListType.X, op=ALU.add)

            def bn_post(ccout_sb, g_sb, be_sb, scale_t, bias_t, tag, eps_ap):
                mgx = small.tile([P, 2 * CT], f32, tag=tag + "mgx")
                nc.vector.tensor_scalar_mul(out=mgx[:], in0=ccout_sb[:],
                                            scalar1=1.0 / (B * L))
                mg = mgx[:, 0:CT]
                ex2 = mgx[:, CT:2 * CT]
                nvar = small.tile([P, CT], f32, tag=tag + "nv")
                # nvar = mean^2 - E[x^2] = -var
                nc.vector.tensor_tensor(out=nvar[:], in0=mg, in1=mg, op=ALU.mult)
                nc.vector.tensor_tensor(out=nvar[:], in0=nvar[:], in1=ex2, op=ALU.subtract)
                sd = small.tile([P, CT], f32, tag=tag + "sd")
                nc.scalar.activation(out=sd[:], in_=nvar[:], func=AF.Sqrt,
                                     scale=-1.0, bias=eps_ap)
                rstd = small.tile([P, CT], f32, tag=tag + "rstd")
                nc.vector.reciprocal(out=rstd[:], in_=sd[:])
                nc.vector.tensor_tensor(out=scale_t[:], in0=rstd[:], in1=g_sb[:], op=ALU.mult)
                tmp = small.tile([P, CT], f32, tag=tag + "tmp")
                nc.vector.tensor_tensor(out=tmp[:], in0=mg, in1=scale_t[:], op=ALU.mult)
                nc.vector.tensor_tensor(out=bias_t[:], in0=be_sb[:], in1=tmp[:], op=ALU.subtract)

            # ---------------- Phase A: attention ----------------
            # kq for BOTH batches first (pipelines b1's staging under b0)
            for b in range(BL):
                for mc in range(MC):
                    ms = slice(mc * 512, (mc + 1) * 512)
                    kps = psum.tile([P, 512], f32, tag="ps")
                    for ct in range(CT):
                        nc.tensor.matmul(
                            out=kps[:],
                            lhsT=wkq_sb[:, ct, :],
                            rhs=x_sb[:, b, ct, ms],
                            start=(ct == 0), stop=(ct == CT - 1))
                    # rows 0:64 keys+bk, 64:128 queries+bq (one DVE op)
                    nc.vector.tensor_scalar_add(out=keys_sb[:, b, ms],
                                                in0=kps[:], scalar1=bkq_sb)
                # queries to partition base 0 (SBUF->SBUF remap)
                nc.gpsimd.dma_start(out=queries_sb[0:A, b, :],
                                    in_=keys_sb[A:2 * A, b, :])
                if b == 0:
                    # gpsimd reaches here only after the b0 queries move:
                    # xT + conv weights now load behind phase-A traffic
                    nc.gpsimd.dma_start(out=xT_tiles[0][:],
                                        in_=xT_ext[0].rearrange("(lc p) c -> p lc c", p=P))
                    nc.gpsimd.dma_start(out=xT_tiles[1][:],
                                        in_=xT_ext[1].rearrange("(lc p) c -> p lc c", p=P))
                    nc.gpsimd.dma_start(out=w1_sb[:],
                                        in_=w1_ext[:].rearrange("p (kc c) -> p kc c", c=C))
                    nc.gpsimd.dma_start(out=w2_sb[:],
                                        in_=w2_ext[:].rearrange("p (kc c) -> p kc c", c=C))
                    nc.gpsimd.dma_start(out=wp_sb[:],
                                        in_=wp_ext[:].rearrange("p (ct o) -> p ct o", ct=CT))

            # per-b stage emitters; interleaved b0/b1 for engine balance
            e_t, rsp_t, rcp_t, xTs_t, ao_t = {}, {}, {}, {}, {}

            def stage_scores(b):
                xT_sb = xT_tiles[b]
                e_sb = pab.tile([P, LT, L], fp8, tag="e", name=f"e{b}")
                rsp = pab.tile([P, LT, MC], f32, tag="rsp", name=f"rsp{b}")
                rcp = pab.tile([P, LT], f32, tag="rcp", name=f"rcp{b}")
                xTs = pab.tile([P, LT, C], fp8, tag="xTs", name=f"xTs{b}")
                e_t[b], rsp_t[b], rcp_t[b], xTs_t[b] = e_sb, rsp, rcp, xTs
                for lc in range(LT):
                    for mc in range(MC):
                        sps = psum.tile([P, 512], f32, tag="ps")
                        nc.tensor.matmul(
                            out=sps[:],
                            lhsT=keys_sb[0:A, b, lc * P:(lc + 1) * P],
                            rhs=queries_sb[0:A, b, mc * 512:(mc + 1) * 512],
                            start=True, stop=True)
                        # exp -> fp8, free row sums via the ACT accumulator
                        nc.scalar.activation(
                            out=e_sb[:, lc, mc * 512:(mc + 1) * 512],
                            in_=sps[:], func=AF.Exp, scale=SM_SCALE,
                            accum_out=rsp[:, lc, mc:mc + 1])
                    nc.vector.scalar_tensor_tensor(
                        out=rcp[:, lc:lc + 1], in0=rsp[:, lc, 0:1],
                        scalar=1.0, in1=rsp[:, lc, 1:2],
                        op0=ALU.mult, op1=ALU.add)
                    nc.vector.reciprocal(out=rcp[:, lc:lc + 1],
                                         in_=rcp[:, lc:lc + 1])
                    # xTs[l, c] = xT[l, c] * (S_A / rowsum[l]) -> fp8
                    nc.vector.tensor_scalar(out=xTs[:, lc, :],
                                            in0=xT_sb[:, lc, :],
                                            scalar1=rcp[:, lc:lc + 1],
                                            scalar2=S_A,
                                            op0=ALU.mult, op1=ALU.mult)

            def stage_attnout(b):
                e_sb, xTs = e_t[b], xTs_t[b]
                ao_sb = pab.tile([P, CT, L], fp8, tag="ao", name=f"ao{b}")
                ao_t[b] = ao_sb
                for cc in range(CT):
                    for mc in range(MC):
                        ms = slice(mc * 512, (mc + 1) * 512)
                        aps = psum.tile([P, 512], f32, tag="ps")
                        for lcp in range(LT // 2):
                            nc.tensor.matmul(
                                out=aps[:],
                                lhsT=xTs[:, 2 * lcp:2 * lcp + 2, cc * P:(cc + 1) * P],
                                rhs=e_sb[:, 2 * lcp:2 * lcp + 2, ms],
                                start=(lcp == 0), stop=(lcp == LT // 2 - 1),
                                perf_mode=DR)
                        # b0 casts on DVE, b1 casts on ACT (wave balance)
                        if b == 0:
                            nc.vector.tensor_copy(out=ao_sb[:, cc, ms], in_=aps[:])
                        else:
                            nc.scalar.activation(out=ao_sb[:, cc, ms],
                                                 in_=aps[:], func=AF.Copy)

            def stage_proj(b):
                ao_sb = ao_t[b]
                for oc in range(CT):
                    for mc in range(MC):
                        ms = slice(mc * 512, (mc + 1) * 512)
                        pps = psum.tile([P, 512], f32, tag="ps")
                        for cp in range(CT // 2):
                            nc.tensor.matmul(
                                out=pps[:],
                                lhsT=wp_sb[:, 2 * cp:2 * cp + 2, oc * P:(oc + 1) * P],
                                rhs=ao_sb[:, 2 * cp:2 * cp + 2, ms],
                                start=(cp == 0), stop=(cp == CT // 2 - 1),
                                perf_mode=DR)
                        # x2' = psum/(S_W*S_A) + x  (bp deferred: BN1 is
                        # invariant to per-channel constants; bp rejoins in
                        # the conv2 epilogue via b2+bp)
                        nc.vector.scalar_tensor_tensor(
                            out=x2_sb[:, b, oc, ms], in0=pps[:],
                            scalar=1.0 / (S_W * S_A), in1=x_sb[:, b, oc, ms],
                            op0=ALU.mult, op1=ALU.add,
                            accum_out=m1a[:, oc, 2 * b + mc:2 * b + mc + 1])
                        sqs = ostage.tile([P, 512], f32, tag="sqs")
                        nc.scalar.activation(
                            out=sqs[:], in_=x2_sb[:, b, oc, ms], func=AF.Square,
                            accum_out=m2a[:, oc, 2 * b + mc:2 * b + mc + 1])

            stage_scores(0)
            stage_attnout(0)
            stage_scores(1)
            stage_proj(0)
            stage_attnout(1)
            stage_proj(1)

            # ---------------- BN1 + conv1 ----------------
            pack_stats(m1a, m2a, ccin1_sb)
            stats_allreduce(cc1_in, cc1_out, ccin1_sb, ccout1_sb, g1g_sb)
            for fn in (AF.Sqrt, AF.Relu):   # re-warm while the mesh runs
                nc.scalar.activation(out=warm[:], in_=eps_sb[:, 0:1], func=fn)
            bn_post(ccout1_sb, g1_sb, be1_sb, scale1, bias1, "p1", eps_sb[:, 0:1])

            # h = relu(bn1(x2)), padded. Emitted as first-halves (in the
            # order conv1's hc=0 consumes them) then second-halves: the
            # first DR accumulation unblocks after ~2 ops, and all of hc=0
            # needs only first halves.
            RORD = [(0, 0), (0, 1), (1, 0), (1, 1), (0, 2), (0, 3), (1, 2), (1, 3)]
            for b, ct in RORD:
                nc.scalar.activation(out=h_sb[:, b, ct, 1:515],
                                     in_=x2_sb[:, b, ct, 0:514],
                                     func=AF.Relu,
                                     scale=scale1[:, ct:ct + 1],
                                     bias=bias1[:, ct:ct + 1])
            for b, ct in RORD:
                nc.scalar.activation(out=h_sb[:, b, ct, 515:L + 1],
                                     in_=x2_sb[:, b, ct, 514:L],
                                     func=AF.Relu,
                                     scale=scale1[:, ct:ct + 1],
                                     bias=bias1[:, ct:ct + 1])

            # conv1: hc-outer, oc-inner, 2 psum banks (b)
            for hc in range(MC):
                for oc in range(CT):
                    cps = [psum.tile([P, 512], f32, tag="ps", name=f"c1ps{hc}_{oc}_{_j}")
                           for _j in range(BL)]
                    if CONV1_FP8:
                        for cp in range(CT // 2):
                            for k in range(3):
                                w_ap = w1_sb[:, k * CT + 2 * cp:k * CT + 2 * cp + 2,
                                             oc * P:(oc + 1) * P]
                                for b in range(BL):
                                    nc.tensor.matmul(
                                        out=cps[b][:], lhsT=w_ap,
                                        rhs=h_sb[:, b, 2 * cp:2 * cp + 2,
                                                 hc * 512 + k:hc * 512 + k + 512],
                                        start=(cp == 0 and k == 0),
                                        stop=(cp == CT // 2 - 1 and k == 2),
                                        perf_mode=DR)
                    else:
                        for ct in range(CT):
                            for k in range(3):
                                w_ap = w1_sb[:, k * CT + ct, oc * P:(oc + 1) * P]
                                for b in range(BL):
                                    nc.tensor.matmul(
                                        out=cps[b][:], lhsT=w_ap,
                                        rhs=h_sb[:, b, ct, hc * 512 + k:hc * 512 + k + 512],
                                        start=(ct == 0 and k == 0),
                                        stop=(ct == CT - 1 and k == 2))
                    for b in range(BL):
                        hs = slice(hc * 512, (hc + 1) * 512)
                        nc.vector.tensor_scalar(
                            out=h2_sb[:, b, oc, hs], in0=cps[b][:],
                            scalar1=b1_sb[:, oc:oc + 1], scalar2=0.0,
                            op0=ALU.add, op1=ALU.add,
                            accum_out=n1a[:, oc, 2 * b + hc:2 * b + hc + 1])
                        sqs = ostage.tile([P, 512], f32, tag="sqs")
                        nc.scalar.activation(
                            out=sqs[:], in_=h2_sb[:, b, oc, hs], func=AF.Square,
                            accum_out=n2a[:, oc, 2 * b + hc:2 * b + hc + 1])

            # ---------------- BN2 + conv2 ----------------
            pack_stats(n1a, n2a, ccin2_sb)
            stats_allreduce(cc2_in, cc2_out, ccin2_sb, ccout2_sb, g2g_sb)
            warm2 = (AF.Sqrt, AF.Relu, AF.Identity) if CONV2_FP8 else (AF.Sqrt, AF.Relu)
            for fn in warm2:                # re-warm while the mesh runs
                nc.scalar.activation(out=warm[:], in_=eps_sb[:, 0:1], func=fn)
            bn_post(ccout2_sb, g2_sb, be2_sb, scale2, bias2, "p2", eps_sb[:, 1:2])

            # h3 = relu(bn2(h2)), same half-ordering as h
            for b, ct in RORD:
                nc.scalar.activation(out=h3_sb[:, b, ct, 1:515],
                                     in_=h2_sb[:, b, ct, 0:514],
                                     func=AF.Relu,
                                     scale=scale2[:, ct:ct + 1],
                                     bias=bias2[:, ct:ct + 1])
            for b, ct in RORD:
                nc.scalar.activation(out=h3_sb[:, b, ct, 515:L + 1],
                                     in_=h2_sb[:, b, ct, 514:L],
                                     func=AF.Relu,
                                     scale=scale2[:, ct:ct + 1],
                                     bias=bias2[:, ct:ct + 1])

            # conv2 + b2 + residual -> out, streaming 512-col chunks to HBM
            for hc in range(MC):
                for oc in range(CT):
                    cps = [psum.tile([P, 512], f32, tag="ps", name=f"c2ps{hc}_{oc}_{_j}")
                           for _j in range(BL)]
                    if CONV2_FP8:
                        for cp in range(CT // 2):
                            for k in range(3):
                                w_ap = w2_sb[:, k * CT + 2 * cp:k * CT + 2 * cp + 2,
                                             oc * P:(oc + 1) * P]
                                for b in range(BL):
                                    nc.tensor.matmul(
                                        out=cps[b][:], lhsT=w_ap,
                                        rhs=h3_sb[:, b, 2 * cp:2 * cp + 2,
                                                  hc * 512 + k:hc * 512 + k + 512],
                                        start=(cp == 0 and k == 0),
                                        stop=(cp == CT // 2 - 1 and k == 2),
                                        perf_mode=DR)
                    else:
                        for ct in range(CT):
                            for k in range(3):
                                w_ap = w2_sb[:, k * CT + ct, oc * P:(oc + 1) * P]
                                for b in range(BL):
                                    nc.tensor.matmul(
                                        out=cps[b][:], lhsT=w_ap,
                                        rhs=h3_sb[:, b, ct, hc * 512 + k:hc * 512 + k + 512],
                                        start=(ct == 0 and k == 0),
                                        stop=(ct == CT - 1 and k == 2))
                    for b in range(BL):
                        hs = slice(hc * 512, (hc + 1) * 512)
                        og = ostage.tile([P, 512], f32, tag="og")
                        if CONV2_FP8:
                            ogt = ostage.tile([P, 512], f32, tag="ogt")
                            nc.scalar.activation(
                                out=ogt[:], in_=cps[b][:], func=AF.Identity,
                                scale=1.0 / S_W, bias=b2_sb[:, oc:oc + 1])
                            nc.vector.tensor_tensor(
                                out=og[:], in0=ogt[:],
                                in1=x2_sb[:, b, oc, hs], op=ALU.add)
                        else:
                            nc.vector.scalar_tensor_tensor(
                                out=og[:], in0=cps[b][:],
                                scalar=b2_sb[:, oc:oc + 1],
                                in1=x2_sb[:, b, oc, hs],
                                op0=ALU.add, op1=ALU.add)
                        nc.sync.dma_start(
                            out=out_ext[b, oc * P:(oc + 1) * P, hs], in_=og[:])

    nc.compile()
    return nc


def _get_nc():
    if "nc" not in _CACHE:
        _CACHE["nc"] = _build()
    return _CACHE["nc"]


def _prep_in_maps(inputs):
    import ml_dtypes
    f = np.float32
    bf = ml_dtypes.bfloat16
    f8 = ml_dtypes.float8_e4m3
    x = np.ascontiguousarray(inputs["x"], dtype=f)

    def vec_pct(v):
        # (C,) -> [P, CT] with channel c = ct*P + p at [p, ct]
        return np.asarray(v, dtype=f).reshape(CT, P).T

    pvec = np.concatenate(
        [vec_pct(inputs["bp"]),
         vec_pct(inputs["b1"]) * (S_W if CONV1_FP8 else 1.0),
         vec_pct(inputs["b2"]) + vec_pct(inputs["bp"]),
         vec_pct(inputs["g1"]), vec_pct(inputs["be1"]),
         vec_pct(inputs["g2"]), vec_pct(inputs["be2"]),
         np.concatenate([inputs["bk"], inputs["bq"]]).reshape(P, 1).astype(f)],
        axis=1)

    def swiz2(w):  # [C, X] -> [P, CT*X] partition-major
        X = w.shape[1]
        return np.ascontiguousarray(
            w.reshape(CT, P, X).transpose(1, 0, 2).reshape(P, CT * X))

    def swiz3(w):  # [3, C, C] (k, i, o) -> [P, 3*CT*C] with cols (k*CT+ct)*C+o
        return np.ascontiguousarray(
            w.reshape(3, CT, P, C).transpose(2, 0, 1, 3).reshape(P, 3 * CT * C))

    shared = {
        "wkq": swiz2(np.concatenate([inputs["Wk"].T, inputs["Wq"].T], axis=1).astype(bf)),
        "wp": swiz2((inputs["Wp"].T * S_W).astype(f8)),
        "w1": swiz3((np.transpose(inputs["W1"], (2, 1, 0)) * S_W).astype(f8)
                    if CONV1_FP8 else
                    np.transpose(inputs["W1"], (2, 1, 0)).astype(bf)),
        "w2": swiz3((np.transpose(inputs["W2"], (2, 1, 0)) * S_W).astype(f8)
                    if CONV2_FP8 else
                    np.transpose(inputs["W2"], (2, 1, 0)).astype(bf)),
        "pvec": np.ascontiguousarray(pvec, dtype=f),
    }
    in_maps = []
    for i in range(NCORES):
        xl = np.ascontiguousarray(x[i * BL:(i + 1) * BL])
        xTl = np.ascontiguousarray(np.transpose(xl, (0, 2, 1)).astype(bf))
        m = {"x": xl.astype(bf), "xT": xTl}
        m.update(shared)
        in_maps.append(m)
    return in_maps


def kernel(**inputs) -> np.ndarray:
    from concourse import bass_utils
    nc = _get_nc()
    in_maps = _prep_in_maps(inputs)
    res = bass_utils.run_bass_kernel_spmd(nc, in_maps, list(range(NCORES)))
    return np.concatenate([r["out"] for r in res.results], axis=0)
